# revision 1
# baseline (speedup 1.0000x reference)
# Trainium2 Bass kernel for AttentionWithSink
# B=2, S=2048, D=1024, H=16 heads (hd=64), 8 sink tokens, full bidirectional
# attention over T=2056 tokens, output projection back to D.
#
# Sharding: 8 cores = 2 batches x 4 head-groups (4 heads each).
# Each core computes QKV for its 4 heads over its batch, transposed-scores
# attention (keys on partitions => no transposes anywhere), and a partial
# output projection over its 256 head-dims. Host sums the 4 partials per
# batch (tensor-parallel unshard).
import numpy as np

B, S, D, H, HD, NS = 2, 2048, 1024, 16, 64, 8
T = S + NS            # 2056 tokens incl. sinks (sinks stored LAST)
NCORES = 8
HPG = 4               # heads per group/core
GD = HPG * HD         # 256 head-dims per core
NKC = 17              # key chunks: 16*128 + 8
KREM = T - 16 * 128   # 8
NQC = 4               # query chunks
QCH = 512
VBLK = 130            # v' block: vA(64)|1|vB(64)|1

_prog_cache = {}
ATT_BF16 = True


def _emit_body(nc, tc, tile, mybir, dr, pers, parts="baevdy"):
    """One full pass: QKV projections + attention + out-projection."""
    F32R = mybir.dt.float32r
    F32 = mybir.dt.float32
    VDT = mybir.dt.bfloat16 if ATT_BF16 else F32R
    AF = mybir.ActivationFunctionType
    ALU = mybir.AluOpType
    qT, kT, vp, wo_sb, bqt, bkt, bv_bc, bo_bc, ones64 = pers

    if "b" in parts:
      # ---------------- Phase B: QKV projections ----------------
      with (
        tc.tile_pool(name="xw", bufs=1) as xw,
        tc.tile_pool(name="psqk", bufs=2, space="PSUM") as psb,
        tc.tile_pool(name="psv", bufs=2, space="PSUM") as psv,
    ):
        xt = [xw.tile([128, T], F32R, tag=f"x{dd}", name=f"x{dd}") for dd in range(8)]
        _eng = [nc.sync, nc.scalar, nc.gpsimd]
        _ei = 0
        CSP = 514
        for dd in range(8):
            for cc in range(4):
                c0, c1 = cc * CSP, min((cc + 1) * CSP, T)
                _eng[_ei % 3].dma_start(
                    xt[dd][:, c0:c1], dr["xT"][dd * 128 : (dd + 1) * 128, c0:c1]
                )
                _ei += 1
        w_sb = {}
        for nm in ("q", "k", "v"):
            w = xw.tile([128, 8 * GD], F32R, tag=f"w{nm}", name=f"w{nm}")
            for dd in range(8):
                _eng[_ei % 3].dma_start(
                    w[:, dd * GD : (dd + 1) * GD],
                    dr[f"w{nm}_t"][dd * 128 : (dd + 1) * 128, :],
                )
                _ei += 1
            w_sb[nm] = w

        def emit_v():
            for tcx in range(NKC):
                kk = 128 if tcx < 16 else KREM
                pv = psv.tile([128, GD], F32, tag="pv", name=f"pv{tcx}")
                for dd in range(8):
                    nc.tensor.matmul(
                        pv[:kk, :],
                        xt[dd][:, tcx * 128 : tcx * 128 + kk],
                        w_sb["v"][:, dd * GD : (dd + 1) * GD],
                        start=(dd == 0),
                        stop=(dd == 7),
                    )
                c0 = tcx * VBLK
                for pr in range(2):
                    for hh in range(2):
                        h = pr * 2 + hh
                        nc.vector.tensor_tensor(
                            vp[pr][:kk, c0 + hh * 65 : c0 + hh * 65 + 64],
                            pv[:kk, h * 64 : (h + 1) * 64],
                            bv_bc[:kk, h * 64 : (h + 1) * 64],
                            op=ALU.add,
                        )

        def emit_q(qcs):
            # dd-outer: one weight load per (i, dd), reused across query chunks
            for i in range(2):
                pqs = {qc: psb.tile([128, QCH], F32, tag="pq", name=f"pq{i}_{qc}", bufs=5)
                       for qc in qcs}
                for dd in range(8):
                    for qc in qcs:
                        nc.tensor.matmul(
                            pqs[qc][:],
                            w_sb["q"][:, dd * GD + i * 128 : dd * GD + i * 128 + 128],
                            xt[dd][:, qc * QCH : (qc + 1) * QCH],
                            start=(dd == 0),
                            stop=(dd == 7),
                        )
                for qc in qcs:
                    nc.vector.tensor_scalar_add(
                        qT[i][:, qc * QCH : (qc + 1) * QCH], pqs[qc][:], bqt[i][:, 0:1]
                    )

        def emit_k():
            for i in range(2):
                pks = {kc5: psb.tile([128, QCH], F32, tag="pq", name=f"pk{i}_{kc5}", bufs=5)
                       for kc5 in range(5)}
                for dd in range(8):
                    for kc5 in range(5):
                        n = QCH if kc5 < 4 else KREM
                        nc.tensor.matmul(
                            pks[kc5][:, :n],
                            w_sb["k"][:, dd * GD + i * 128 : dd * GD + i * 128 + 128],
                            xt[dd][:, kc5 * QCH : kc5 * QCH + n],
                            start=(dd == 0),
                            stop=(dd == 7),
                        )
                for kc5 in range(5):
                    n = QCH if kc5 < 4 else KREM
                    nc.vector.tensor_scalar_add(
                        kT[i][:, kc5 * QCH : kc5 * QCH + n], pks[kc5][:, :n], bkt[i][:, 0:1]
                    )

        emit_k()
        emit_q([0])
        emit_v()
        emit_q([1, 2, 3])

    if "a" not in parts:
        return
    # ---------------- Phase C/D: attention + out-projection ----------------
    with (
        tc.tile_pool(name="sc", bufs=2, space="PSUM") as scp,
        tc.tile_pool(name="av", bufs=2, space="PSUM") as avp,
        tc.tile_pool(name="py", bufs=2, space="PSUM") as pyp,
        tc.tile_pool(name="pt", bufs=4) as ptp,
        tc.tile_pool(name="onT", bufs=4) as onp,
        tc.tile_pool(name="ysb", bufs=3) as yp,
        tc.tile_pool(name="small", bufs=2) as sp,
    ):
        def emit_y_unit(pqc, onTs_prev, ts_, dc):
            py = pyp.tile([128, QCH], F32, tag="py", name=f"py_{pqc}_{ts_}_{dc}")
            nc.tensor.matmul(
                py[:], onTs_prev[0][:, ts_ * 128 : (ts_ + 1) * 128],
                wo_sb[0][:, dc * QCH : (dc + 1) * QCH],
                start=True, stop=False,
            )
            nc.tensor.matmul(
                py[:], onTs_prev[1][:, ts_ * 128 : (ts_ + 1) * 128],
                wo_sb[1][:, dc * QCH : (dc + 1) * QCH],
                start=False, stop=True,
            )
            ys = yp.tile([128, QCH], F32, tag="ys", name=f"ys_{pqc}_{ts_}_{dc}")
            nc.vector.tensor_tensor(
                ys[:], py[:], bo_bc[:, dc * QCH : (dc + 1) * QCH],
                op=ALU.add,
            )
            nc.sync.dma_start(
                dr["y"][pqc * QCH + ts_ * 128 : pqc * QCH + (ts_ + 1) * 128,
                        dc * QCH : (dc + 1) * QCH],
                ys[:],
            )

        do_y = "y" in parts and "v" in parts
        pending = None
        for qc in range(NQC):
            onTs = []
            for pr in range(2):
                VA = avp.tile([128, QCH], F32, tag="av", name=f"VA_{qc}_{pr}")
                VB = avp.tile([128, QCH], F32, tag="av", name=f"VB_{qc}_{pr}")
                for kc in range(NKC):
                    kk = 128 if kc < 16 else KREM
                    Sps = scp.tile([128, 2 * QCH], F32, tag="s", name=f"S_{qc}_{pr}_{kc}")
                    PT = ptp.tile([128, 2 * QCH], VDT, tag="pt", name=f"PT_{qc}_{pr}_{kc}")
                    nc.tensor.matmul(
                        Sps[:kk, 0:QCH],
                        kT[pr][0:64, kc * 128 : kc * 128 + kk],
                        qT[pr][0:64, qc * QCH : (qc + 1) * QCH],
                        start=True,
                        stop=True,
                    )
                    nc.tensor.matmul(
                        Sps[:kk, QCH : 2 * QCH],
                        kT[pr][64:128, kc * 128 : kc * 128 + kk],
                        qT[pr][64:128, qc * QCH : (qc + 1) * QCH],
                        start=True,
                        stop=True,
                    )
                    if "e" in parts:
                        nc.scalar.activation(PT[:kk, :], Sps[:kk, :], AF.Exp)
                    st, stp = kc == 0, kc == NKC - 1
                    c0 = kc * VBLK
                    if "v" in parts:
                        nc.tensor.matmul(
                            VA[0:65, :], vp[pr][:kk, c0 : c0 + 65],
                            PT[:kk, 0:QCH], start=st, stop=stp,
                        )
                        nc.tensor.matmul(
                            VB[0:65, :], vp[pr][:kk, c0 + 65 : c0 + VBLK],
                            PT[:kk, QCH : 2 * QCH], start=st, stop=stp,
                        )
                    if do_y and pr == 0 and pending is not None and kc % 2 == 1 and (kc - 1) // 2 < 8:
                        u = (kc - 1) // 2
                        emit_y_unit(pending[0], pending[1], u // 2, u % 2)
                if "v" not in parts:
                    continue
                onT = onp.tile([128, QCH], F32R, tag="onT", name=f"onT_{qc}_{pr}")
                for hh, V in ((0, VA), (1, VB)):
                    if "d" in parts:
                        rc = sp.tile([1, QCH], F32R, tag="rc", name=f"rc_{qc}_{pr}_{hh}")
                        with nc.allow_low_precision(reason="f32r is bit-compatible fp32"):
                            nc.vector.reciprocal(rc[:], V[64:65, :])
                        bc = pyp.tile([64, QCH], F32, tag="py", name=f"bc_{qc}_{pr}_{hh}")
                        nc.tensor.matmul(bc[:], ones64[:], rc[:], start=True, stop=True)
                        bcs = sp.tile([64, QCH], F32, tag="bcs", name=f"bcs_{qc}_{pr}_{hh}")
                        nc.vector.tensor_copy(bcs[:], bc[:])
                        nc.vector.tensor_tensor(
                            onT[hh * 64 : hh * 64 + 64, :],
                            V[0:64, :], bcs[:], op=ALU.mult,
                        )
                    else:
                        nc.vector.tensor_copy(onT[hh * 64 : hh * 64 + 64, :], V[0:64, :])
                onTs.append(onT)
            if do_y:
                pending = (qc, onTs)
        if do_y and pending is not None:
            for u in range(8):
                emit_y_unit(pending[0], pending[1], u // 2, u % 2)


def _build_program(reps=1, parts="baevdy"):
    import concourse.bass as bass  # noqa: F401
    import concourse.mybir as mybir
    import concourse.tile as tile
    from concourse import bacc

    F32R = mybir.dt.float32r
    F32 = mybir.dt.float32

    nc = bacc.Bacc("TRN2", num_devices=NCORES)
    dr = {
        "xT": nc.dram_tensor("xT", [D, T], F32R, kind="ExternalInput"),
        "wq_t": nc.dram_tensor("wq_t", [D, GD], F32R, kind="ExternalInput"),
        "wk_t": nc.dram_tensor("wk_t", [D, GD], F32R, kind="ExternalInput"),
        "wv_t": nc.dram_tensor("wv_t", [D, GD], F32R, kind="ExternalInput"),
        "wo_t": nc.dram_tensor("wo_t", [GD, D], F32R, kind="ExternalInput"),
        "bq": nc.dram_tensor("bq", [GD, 1], F32, kind="ExternalInput"),
        "bk": nc.dram_tensor("bk", [GD, 1], F32, kind="ExternalInput"),
        "bv": nc.dram_tensor("bv", [1, GD], F32, kind="ExternalInput"),
        "bo": nc.dram_tensor("bo", [1, D], F32, kind="ExternalInput"),
        "y": nc.dram_tensor("y", [S, D], F32, kind="ExternalOutput"),
    }

    with tile.TileContext(nc) as tc:
        with tc.tile_pool(name="persist", bufs=1) as pp:
            VDT = mybir.dt.bfloat16 if ATT_BF16 else F32R
            qT = [pp.tile([128, S], F32R, tag=f"qT{i}", name=f"qT{i}") for i in range(2)]
            kT = [pp.tile([128, T], F32R, tag=f"kT{i}", name=f"kT{i}") for i in range(2)]
            vp = [pp.tile([128, NKC * VBLK], VDT, tag=f"vp{i}", name=f"vp{i}") for i in range(2)]
            wo_sb = [pp.tile([128, D], F32R, tag=f"wo{i}", name=f"wo{i}") for i in range(2)]
            bqt = [pp.tile([128, 1], F32, tag=f"bq{i}", name=f"bq{i}") for i in range(2)]
            bkt = [pp.tile([128, 1], F32, tag=f"bk{i}", name=f"bk{i}") for i in range(2)]
            for i in range(2):
                nc.sync.dma_start(wo_sb[i][:], dr["wo_t"][i * 128 : (i + 1) * 128, :])
                nc.sync.dma_start(bqt[i][:], dr["bq"][i * 128 : (i + 1) * 128, :])
                nc.sync.dma_start(bkt[i][:], dr["bk"][i * 128 : (i + 1) * 128, :])
            bv_sb = pp.tile([1, GD], F32, tag="bv")
            bo_sb = pp.tile([1, D], F32, tag="bo")
            nc.sync.dma_start(bv_sb[:], dr["bv"][:])
            nc.sync.dma_start(bo_sb[:], dr["bo"][:])
            bv_bc = pp.tile([128, GD], F32, tag="bvbc")
            bo_bc = pp.tile([128, D], F32, tag="bobc")
            nc.gpsimd.partition_broadcast(bv_bc[:], bv_sb[:])
            nc.gpsimd.partition_broadcast(bo_bc[:], bo_sb[:])
            # ones columns of v' (value columns overwritten later)
            for i in range(2):
                if ATT_BF16:
                    nc.vector.memset(vp[i][:], 1.0)
                else:
                    nc.vector.memset(vp[i][:].bitcast(F32), 1.0)

            if "b" not in parts:
                for i in range(2):
                    nc.vector.memset(qT[i][:].bitcast(F32), 0.0)
                    nc.vector.memset(kT[i][:].bitcast(F32), 0.0)
            ones64_f = pp.tile([1, 64], F32, tag="ones64f")
            nc.vector.memset(ones64_f[:], 1.0)
            ones64 = pp.tile([1, 64], F32R, tag="ones64")
            nc.vector.tensor_copy(ones64[:], ones64_f[:])
            pers = (qT, kT, vp, wo_sb, bqt, bkt, bv_bc, bo_bc, ones64)
            for _rep in range(reps):
                _emit_body(nc, tc, tile, mybir, dr, pers, parts)
    nc.compile()
    return nc


def _get_program(reps=1, parts="baevdy"):
    key = f"nc{reps}_{parts}_{ATT_BF16}"
    if key not in _prog_cache:
        _prog_cache[key] = _build_program(reps, parts)
    return _prog_cache[key]


def _host_inputs(x, sink_tokens, wq, bq, wk, bk, wv, bv, wo, bo):
    f = np.float32
    x = np.asarray(x, f)
    sink = np.asarray(sink_tokens, f)[0]            # [NS, D]
    wq, wk, wv, wo = (np.asarray(a, f) for a in (wq, wk, wv, wo))
    bq, bk, bv, bo = (np.asarray(a, f) for a in (bq, bk, bv, bo))
    sc = np.float32(1.0 / np.sqrt(HD))
    in_maps = []
    for core in range(NCORES):
        b, g = core // 4, core % 4
        xs = np.concatenate([x[b], sink], axis=0)   # sinks LAST
        xT = np.ascontiguousarray(xs.T)
        sl = slice(g * GD, (g + 1) * GD)
        in_maps.append({
            "xT": xT,
            "wq_t": np.ascontiguousarray(wq[sl].T) * sc,
            "wk_t": np.ascontiguousarray(wk[sl].T),
            "wv_t": np.ascontiguousarray(wv[sl].T),
            "wo_t": np.ascontiguousarray(wo[:, sl].T),
            "bq": (bq[sl] * sc).reshape(GD, 1).copy(),
            "bk": bk[sl].reshape(GD, 1).copy(),
            "bv": bv[sl].reshape(1, GD).copy(),
            "bo": (bo if g == 0 else np.zeros_like(bo)).reshape(1, D).copy(),
        })
    return in_maps


def kernel(x, sink_tokens, wq, bq, wk, bk, wv, bv, wo, bo):
    from concourse.bass_utils import run_bass_kernel_spmd

    nc = _get_program()
    in_maps = _host_inputs(x, sink_tokens, wq, bq, wk, bk, wv, bv, wo, bo)
    res = None
    last_exc = None
    for attempt in range(3):
        try:
            res = run_bass_kernel_spmd(nc, in_maps, core_ids=list(range(NCORES)))
            break
        except Exception as e:  # transient NRT/axon failures: retry
            last_exc = e
            import time as _time
            _time.sleep(2.0 * (attempt + 1))
    if res is None:
        raise last_exc
    y = np.zeros((B, S, D), np.float64)
    for core in range(NCORES):
        y[core // 4] += res.results[core]["y"]
    return y.astype(np.float32)



# revision 17
# speedup vs baseline: 1391.8995x; 1391.8995x over previous
# Trainium2 Bass kernel for AttentionWithSink
# B=2, S=2048, D=1024, H=16 heads (hd=64), 8 sink tokens, full bidirectional
# attention over T=2056 tokens, output projection back to D.
#
# Sharding: 8 cores = 2 batches x 4 head-groups (4 heads each).
# Each core computes QKV for its 4 heads over its batch, transposed-scores
# attention (keys on partitions => no transposes anywhere), and a partial
# output projection over its 256 head-dims. Host sums the 4 partials per
# batch (tensor-parallel unshard).
#
# v2: fp16 operands end-to-end (PSUM accumulation stays fp32), engine
# rebalance: exp on ACT except sink chunk (Schraudolph on DVE), q/k
# bias+PSUM-drain on ACT, v assembly on Pool, softmax normalization via
# reciprocal_approx_fast + Pool partition_broadcast, out-proj bias folded
# into an extra f32r matmul, 4KB-row y DMAs.
import numpy as np

B, S, D, H, HD, NS = 2, 2048, 1024, 16, 64, 8
T = S + NS            # 2056 tokens incl. sinks (sinks stored LAST)
NCORES = 8
HPG = 4               # heads per group/core
GD = HPG * HD         # 256 head-dims per core
NKC = 17              # key chunks: 16*128 + 8
KREM = T - 16 * 128   # 8
NQC = 4               # query chunks
QCH = 512
VBLK = 130            # v' block: vA(64)|1|vB(64)|1

# Schraudolph fp16 exp: i16 = in*SCHRAU_A + SCHRAU_B, bitcast to fp16.
SCHRAU_A = float(2**10 / np.log(2.0))
SCHRAU_C = 0.043677448 * 2**10
SCHRAU_B = float(15 * 2**10 - SCHRAU_C)
# number of full 128-key chunks (out of 16) exp'd on DVE via Schraudolph
# (the 8-key sink chunk is always on DVE)
N_SCHRAU = 0
RECIP_FAST = True
PB_BCAST = True

_prog_cache = {}


def _emit_body(nc, tc, tile, mybir, dr, pers):
    F32R = mybir.dt.float32r
    F32 = mybir.dt.float32
    F16 = mybir.dt.float16
    I16 = mybir.dt.int16
    AF = mybir.ActivationFunctionType
    ALU = mybir.AluOpType
    qT, kT, vp, wo_sb, bqt, bkt, bv_bc, bo_r, ones_r = pers

    with (
        tc.tile_pool(name="xw", bufs=1) as xw,
        tc.tile_pool(name="ps", bufs=2, space="PSUM") as psb,
        tc.tile_pool(name="pt", bufs=4) as ptp,
        tc.tile_pool(name="onT", bufs=4) as onp,
        tc.tile_pool(name="ysb", bufs=3) as ysp,
        tc.tile_pool(name="small", bufs=4) as sp,
    ):
        scp = avp = pyp = psv = psb
        # ---------------- input DMAs ----------------
        xt = [xw.tile([128, T], F16, tag=f"x{dd}", name=f"x{dd}") for dd in range(8)]
        w_sb = {}
        for nm, eng in (("k", nc.sync), ("q", nc.gpsimd), ("v", nc.scalar)):
            w = xw.tile([128, 8 * GD], F16, tag=f"w{nm}", name=f"w{nm}")
            eng.dma_start(w[:], dr[f"w{nm}_p"][:])
            w_sb[nm] = w
        _eng = [nc.sync, nc.gpsimd, nc.scalar]
        _ei = 0
        CSP = 1028
        for cc in range(2):
            c0, c1 = cc * CSP, min((cc + 1) * CSP, T)
            for dd in range(8):
                _eng[_ei % 3].dma_start(
                    xt[dd][:, c0:c1], dr["xT"][dd * 128 : (dd + 1) * 128, c0:c1]
                )
                _ei += 1
        for i in range(2):
            nc.gpsimd.dma_start(wo_sb[i][:], dr["wo_t"][i * 128 : (i + 1) * 128, :])

        # ---------------- projections ----------------
        def emit_k():
            # kT[i][gd, tok] = wk_i.T @ x ; bias+drain on ACT (idle in this phase)
            for kc5 in range(5):
                for i in range(2):
                    n = QCH if kc5 < 4 else KREM
                    pk = psb.tile([128, QCH], F32, tag="pq", name=f"pk{i}_{kc5}")
                    for dd in range(8):
                        nc.tensor.matmul(
                            pk[:, :n],
                            w_sb["k"][:, dd * GD + i * 128 : dd * GD + i * 128 + 128],
                            xt[dd][:, kc5 * QCH : kc5 * QCH + n],
                            start=(dd == 0),
                            stop=(dd == 7),
                        )
                    nc.scalar.activation(
                        kT[i][:, kc5 * QCH : kc5 * QCH + n], pk[:, :n],
                        AF.Identity, bias=bkt[i][:, 0:1],
                    )

        def emit_q(qcs):
            for qc in qcs:
                for i in range(2):
                    pq = psb.tile([128, QCH], F32, tag="pq", name=f"pq{i}_{qc}")
                    for dd in range(8):
                        nc.tensor.matmul(
                            pq[:],
                            w_sb["q"][:, dd * GD + i * 128 : dd * GD + i * 128 + 128],
                            xt[dd][:, qc * QCH : (qc + 1) * QCH],
                            start=(dd == 0),
                            stop=(dd == 7),
                        )
                    nc.scalar.activation(
                        qT[i][:, qc * QCH : (qc + 1) * QCH], pq[:],
                        AF.Identity, bias=bqt[i][:, 0:1],
                    )

        def emit_v_chunk(tcx):
            kk = 128 if tcx < 16 else KREM
            pvt = psv.tile([128, QCH], F32, tag="pq", name=f"pv{tcx}")
            pv = pvt[:, 0:GD]
            for dd in range(8):
                nc.tensor.matmul(
                    pv[:kk, :],
                    xt[dd][:, tcx * 128 : tcx * 128 + kk],
                    w_sb["v"][:, dd * GD : (dd + 1) * GD],
                    start=(dd == 0),
                    stop=(dd == 7),
                )
            c0 = tcx * VBLK
            for pr in range(2):
                for hh in range(2):
                    h = pr * 2 + hh
                    nc.vector.tensor_tensor(
                        vp[pr][:kk, c0 + hh * 65 : c0 + hh * 65 + 64],
                        pv[:kk, h * 64 : (h + 1) * 64],
                        bv_bc[:kk, h * 64 : (h + 1) * 64],
                        op=ALU.add,
                    )

        # ---------------- out-projection ----------------
        def emit_y(qc, onTs):
            # py[q, D] = onT.T @ wo + 1 x bo ; DMA straight from PSUM
            for ts_ in range(4):
                for dc in range(2):
                    py = pyp.tile([128, QCH], F32, tag="pq", name=f"py_{qc}_{ts_}_{dc}")
                    for pr in range(2):
                        nc.tensor.matmul(
                            py[:],
                            onTs[pr][:, ts_ * 128 : (ts_ + 1) * 128],
                            wo_sb[pr][:, dc * QCH : (dc + 1) * QCH],
                            start=(pr == 0), stop=False,
                        )
                    nc.tensor.matmul(
                        py[:],
                        ones_r[:, 0:128],
                        bo_r[:, dc * QCH : (dc + 1) * QCH],
                        start=False, stop=True,
                    )
                    ys = ysp.tile([128, QCH], F32, tag="ys", name=f"ys_{qc}_{ts_}_{dc}")
                    if dc == 0:
                        nc.scalar.activation(ys[:], py[:], AF.Identity)
                    else:
                        nc.vector.tensor_copy(ys[:], py[:])
                    nc.sync.dma_start(
                        dr["y"][qc * QCH + ts_ * 128 : qc * QCH + (ts_ + 1) * 128,
                                dc * QCH : (dc + 1) * QCH],
                        ys[:],
                    )

        # ---------------- attention ----------------
        def att_unit(qc, with_v):
            onTs = []
            for pr in range(2):
                VA = avp.tile([128, QCH], F32, tag="av", name=f"VA_{qc}_{pr}")
                VB = avp.tile([128, QCH], F32, tag="av", name=f"VB_{qc}_{pr}")
                for kc in range(NKC):
                    if with_v and pr == 0:
                        emit_v_chunk(kc)
                    kk = 128 if kc < 16 else KREM
                    Sps = scp.tile([128, 2 * QCH], F32, tag="s", name=f"S_{qc}_{pr}_{kc}")
                    PT = ptp.tile([128, 2 * QCH], F16, tag="pt", name=f"PT_{qc}_{pr}_{kc}")
                    nc.tensor.matmul(
                        Sps[:kk, 0:QCH],
                        kT[pr][0:64, kc * 128 : kc * 128 + kk],
                        qT[pr][0:64, qc * QCH : (qc + 1) * QCH],
                        start=True, stop=True,
                    )
                    nc.tensor.matmul(
                        Sps[:kk, QCH : 2 * QCH],
                        kT[pr][64:128, kc * 128 : kc * 128 + kk],
                        qT[pr][64:128, qc * QCH : (qc + 1) * QCH],
                        start=True, stop=True,
                    )
                    if kc == NKC - 1 or (N_SCHRAU > 0 and kc % (16 // max(N_SCHRAU, 1)) == 2):
                        # Schraudolph exp on DVE (sink chunk + optional tail)
                        with nc.allow_low_precision(reason="fp16 softmax weights"):
                            nc.vector.tensor_scalar(
                                PT[:kk, :].bitcast(I16),
                                Sps[:kk, :],
                                SCHRAU_A, SCHRAU_B,
                                op0=ALU.mult, op1=ALU.add,
                            )
                    else:
                        nc.scalar.activation(PT[:kk, :], Sps[:kk, :], AF.Exp)
                    st, stp = kc == 0, kc == NKC - 1
                    c0 = kc * VBLK
                    nc.tensor.matmul(
                        VA[0:65, :], vp[pr][:kk, c0 : c0 + 65],
                        PT[:kk, 0:QCH], start=st, stop=stp,
                    )
                    nc.tensor.matmul(
                        VB[0:65, :], vp[pr][:kk, c0 + 65 : c0 + VBLK],
                        PT[:kk, QCH : 2 * QCH], start=st, stop=stp,
                    )
                onT = onp.tile([128, QCH], F16, tag="onT", name=f"onT_{qc}_{pr}")
                for hh, V in ((0, VA), (1, VB)):
                    rc = sp.tile([1, QCH], F32, tag="rc", name=f"rc_{qc}_{pr}_{hh}")
                    with nc.allow_low_precision(reason="softmax denom reciprocal"):
                        if RECIP_FAST:
                            dn = sp.tile([1, QCH], F32, tag="dn", name=f"dn_{qc}_{pr}_{hh}")
                            nc.vector.tensor_copy(dn[:], V[64:65, :])
                            nc.vector.reciprocal_approx_fast(rc[:], dn[:])
                        else:
                            nc.vector.reciprocal(rc[:], V[64:65, :])
                    bc = sp.tile([64, QCH], F32, tag="bc", name=f"bc_{qc}_{pr}_{hh}")
                    nc.gpsimd.partition_broadcast(bc[:], rc[:])
                    with nc.allow_low_precision(reason="fp16 attn output"):
                        nc.vector.tensor_tensor(
                            onT[hh * 64 : hh * 64 + 64, :],
                            V[0:64, :], bc[:], op=ALU.mult,
                        )
                onTs.append(onT)
            return onTs

        emit_k()
        emit_q([0])
        y_pend = None
        for qc in range(NQC):
            onTs = att_unit(qc, with_v=(qc == 0))
            if y_pend is not None:
                emit_y(*y_pend)
            y_pend = (qc, onTs)
            if qc == 0:
                emit_q([1, 2, 3])
        emit_y(*y_pend)


def _build_program(reps=1):
    import concourse.bass as bass  # noqa: F401
    import concourse.mybir as mybir
    import concourse.tile as tile
    from concourse import bacc

    F32R = mybir.dt.float32r
    F32 = mybir.dt.float32
    F16 = mybir.dt.float16

    nc = bacc.Bacc("TRN2", num_devices=NCORES)
    dr = {
        "xT": nc.dram_tensor("xT", [D, T], F16, kind="ExternalInput"),
        "wq_p": nc.dram_tensor("wq_p", [128, 8 * GD], F16, kind="ExternalInput"),
        "wk_p": nc.dram_tensor("wk_p", [128, 8 * GD], F16, kind="ExternalInput"),
        "wv_p": nc.dram_tensor("wv_p", [128, 8 * GD], F16, kind="ExternalInput"),
        "wo_t": nc.dram_tensor("wo_t", [GD, D], F16, kind="ExternalInput"),
        "bq": nc.dram_tensor("bq", [GD, 1], F32, kind="ExternalInput"),
        "bk": nc.dram_tensor("bk", [GD, 1], F32, kind="ExternalInput"),
        "bv": nc.dram_tensor("bv", [1, GD], F32, kind="ExternalInput"),
        "bo": nc.dram_tensor("bo", [1, D], F16, kind="ExternalInput"),
        "y": nc.dram_tensor("y", [S, D], F32, kind="ExternalOutput"),
    }

    with tile.TileContext(nc) as tc:
        with tc.tile_pool(name="persist", bufs=1) as pp:
            qT = [pp.tile([128, S], F16, tag=f"qT{i}", name=f"qT{i}") for i in range(2)]
            kT = [pp.tile([128, T], F16, tag=f"kT{i}", name=f"kT{i}") for i in range(2)]
            vp = [pp.tile([128, NKC * VBLK], F16, tag=f"vp{i}", name=f"vp{i}") for i in range(2)]
            wo_sb = [pp.tile([128, D], F16, tag=f"wo{i}", name=f"wo{i}") for i in range(2)]
            bqt = [pp.tile([128, 1], F32, tag=f"bq{i}", name=f"bq{i}") for i in range(2)]
            bkt = [pp.tile([128, 1], F32, tag=f"bk{i}", name=f"bk{i}") for i in range(2)]
            for i in range(2):
                nc.scalar.dma_start(bqt[i][:], dr["bq"][i * 128 : (i + 1) * 128, :])
                nc.scalar.dma_start(bkt[i][:], dr["bk"][i * 128 : (i + 1) * 128, :])
            bv_sb = pp.tile([1, GD], F32, tag="bv")
            nc.scalar.dma_start(bv_sb[:], dr["bv"][:])
            bv_bc = pp.tile([128, GD], F32, tag="bvbc")
            nc.gpsimd.partition_broadcast(bv_bc[:], bv_sb[:])
            # bo + ones row for the bias matmul in the out-projection
            bo_r = pp.tile([1, D], F16, tag="bo")
            nc.scalar.dma_start(bo_r[:], dr["bo"][:])
            ones_r = pp.tile([1, 128], F16, tag="ones_r")
            nc.vector.memset(ones_r[:], 1.0)
            # ones columns of v' (value columns overwritten later)
            for i in range(2):
                nc.vector.memset(vp[i][:], 1.0)
            pers = (qT, kT, vp, wo_sb, bqt, bkt, bv_bc, bo_r, ones_r)
            for _rep in range(reps):
                _emit_body(nc, tc, tile, mybir, dr, pers)
    nc.compile()
    return nc


def _get_program(reps=1):
    key = f"nc{reps}_{N_SCHRAU}_{RECIP_FAST}_{PB_BCAST}"
    if key not in _prog_cache:
        _prog_cache[key] = _build_program(reps)
    return _prog_cache[key]


def _pack(a):
    # [1024, 256] -> [128, 2048]: dd-th 128-row block becomes column block dd
    return np.concatenate([a[dd * 128 : (dd + 1) * 128] for dd in range(8)], axis=1)


def _host_inputs(x, sink_tokens, wq, bq, wk, bk, wv, bv, wo, bo):
    f = np.float32
    h = np.float16
    x = np.asarray(x, f)
    sink = np.asarray(sink_tokens, f)[0]            # [NS, D]
    wq, wk, wv, wo = (np.asarray(a, f) for a in (wq, wk, wv, wo))
    bq, bk, bv, bo = (np.asarray(a, f) for a in (bq, bk, bv, bo))
    sc = np.float32(1.0 / np.sqrt(HD))
    in_maps = []
    for core in range(NCORES):
        b, g = core // 4, core % 4
        xs = np.concatenate([x[b], sink], axis=0)   # sinks LAST
        xT = np.ascontiguousarray(xs.T).astype(h)
        sl = slice(g * GD, (g + 1) * GD)
        in_maps.append({
            "xT": xT,
            "wq_p": _pack(np.ascontiguousarray(wq[sl].T) * sc).astype(h),
            "wk_p": _pack(np.ascontiguousarray(wk[sl].T)).astype(h),
            "wv_p": _pack(np.ascontiguousarray(wv[sl].T)).astype(h),
            "wo_t": np.ascontiguousarray(wo[:, sl].T).astype(h),
            "bq": (bq[sl] * sc).reshape(GD, 1).copy(),
            "bk": bk[sl].reshape(GD, 1).copy(),
            "bv": bv[sl].reshape(1, GD).copy(),
            "bo": (bo if g == 0 else np.zeros_like(bo)).reshape(1, D).astype(h),
        })
    return in_maps


def kernel(x, sink_tokens, wq, bq, wk, bk, wv, bv, wo, bo):
    from concourse.bass_utils import run_bass_kernel_spmd

    nc = _get_program()
    in_maps = _host_inputs(x, sink_tokens, wq, bq, wk, bk, wv, bv, wo, bo)
    res = None
    last_exc = None
    for attempt in range(3):
        try:
            res = run_bass_kernel_spmd(nc, in_maps, core_ids=list(range(NCORES)))
            break
        except Exception as e:  # transient NRT/axon failures: retry
            last_exc = e
            import time as _time
            _time.sleep(2.0 * (attempt + 1))
    if res is None:
        raise last_exc
    y = np.zeros((B, S, D), np.float64)
    for core in range(NCORES):
        y[core // 4] += res.results[core]["y"]
    return y.astype(np.float32)


# revision 18
# speedup vs baseline: 1427.8878x; 1.0259x over previous
# Trainium2 Bass kernel for AttentionWithSink
# B=2, S=2048, D=1024, H=16 heads (hd=64), 8 sink tokens, full bidirectional
# attention over T=2056 tokens, output projection back to D.
#
# Sharding: 8 cores = 2 batches x 4 head-groups (4 heads each).
# Each core computes QKV for its 4 heads over its batch, transposed-scores
# attention (keys on partitions => no transposes anywhere), and a partial
# output projection over its 256 head-dims. Host sums the 4 partials per
# batch (tensor-parallel unshard).
#
# v2: fp16 operands end-to-end (PSUM accumulation stays fp32), engine
# rebalance: exp on ACT except sink chunk (Schraudolph on DVE), q/k
# bias+PSUM-drain on ACT, v assembly on Pool, softmax normalization via
# reciprocal_approx_fast + Pool partition_broadcast, out-proj bias folded
# into an extra f32r matmul, 4KB-row y DMAs.
import numpy as np

B, S, D, H, HD, NS = 2, 2048, 1024, 16, 64, 8
T = S + NS            # 2056 tokens incl. sinks (sinks stored LAST)
NCORES = 8
HPG = 4               # heads per group/core
GD = HPG * HD         # 256 head-dims per core
NKC = 17              # key chunks: 16*128 + 8
KREM = T - 16 * 128   # 8
NQC = 4               # query chunks
QCH = 512
VBLK = 130            # v' block: vA(64)|1|vB(64)|1

# Schraudolph fp16 exp: i16 = in*SCHRAU_A + SCHRAU_B, bitcast to fp16.
SCHRAU_A = float(2**10 / np.log(2.0))
SCHRAU_C = 0.043677448 * 2**10
SCHRAU_B = float(15 * 2**10 - SCHRAU_C)
# number of full 128-key chunks (out of 16) exp'd on DVE via Schraudolph
# (the 8-key sink chunk is always on DVE)
N_SCHRAU = 4
RECIP_FAST = True
PB_BCAST = True

_prog_cache = {}


def _emit_body(nc, tc, tile, mybir, dr, pers):
    F32R = mybir.dt.float32r
    F32 = mybir.dt.float32
    F16 = mybir.dt.float16
    I16 = mybir.dt.int16
    AF = mybir.ActivationFunctionType
    ALU = mybir.AluOpType
    qT, kT, vp, wo_sb, bqt, bkt, bv_bc, bo_r, ones_r = pers

    with (
        tc.tile_pool(name="xw", bufs=1) as xw,
        tc.tile_pool(name="ps", bufs=2, space="PSUM") as psb,
        tc.tile_pool(name="pt", bufs=4) as ptp,
        tc.tile_pool(name="onT", bufs=4) as onp,
        tc.tile_pool(name="ysb", bufs=3) as ysp,
        tc.tile_pool(name="small", bufs=4) as sp,
    ):
        scp = avp = pyp = psv = psb
        # ---------------- input DMAs ----------------
        xt = [xw.tile([128, T], F16, tag=f"x{dd}", name=f"x{dd}") for dd in range(8)]
        w_sb = {}
        for nm, eng in (("k", nc.sync), ("q", nc.gpsimd), ("v", nc.scalar)):
            w = xw.tile([128, 8 * GD], F16, tag=f"w{nm}", name=f"w{nm}")
            eng.dma_start(w[:], dr[f"w{nm}_p"][:])
            w_sb[nm] = w
        _eng = [nc.sync, nc.gpsimd, nc.scalar]
        _ei = 0
        CSP = 1028
        for cc in range(2):
            c0, c1 = cc * CSP, min((cc + 1) * CSP, T)
            for dd in range(8):
                _eng[_ei % 3].dma_start(
                    xt[dd][:, c0:c1], dr["xT"][dd * 128 : (dd + 1) * 128, c0:c1]
                )
                _ei += 1
        for i in range(2):
            nc.gpsimd.dma_start(wo_sb[i][:], dr["wo_t"][i * 128 : (i + 1) * 128, :])

        # ---------------- projections ----------------
        def emit_k():
            # kT[i][gd, tok] = wk_i.T @ x ; bias+drain on ACT (idle in this phase)
            for kc5 in range(5):
                for i in range(2):
                    n = QCH if kc5 < 4 else KREM
                    pk = psb.tile([128, QCH], F32, tag="pq", name=f"pk{i}_{kc5}")
                    for dd in range(8):
                        nc.tensor.matmul(
                            pk[:, :n],
                            w_sb["k"][:, dd * GD + i * 128 : dd * GD + i * 128 + 128],
                            xt[dd][:, kc5 * QCH : kc5 * QCH + n],
                            start=(dd == 0),
                            stop=(dd == 7),
                        )
                    nc.scalar.activation(
                        kT[i][:, kc5 * QCH : kc5 * QCH + n], pk[:, :n],
                        AF.Identity, bias=bkt[i][:, 0:1],
                    )

        def emit_q(qcs):
            for qc in qcs:
                for i in range(2):
                    pq = psb.tile([128, QCH], F32, tag="pq", name=f"pq{i}_{qc}")
                    for dd in range(8):
                        nc.tensor.matmul(
                            pq[:],
                            w_sb["q"][:, dd * GD + i * 128 : dd * GD + i * 128 + 128],
                            xt[dd][:, qc * QCH : (qc + 1) * QCH],
                            start=(dd == 0),
                            stop=(dd == 7),
                        )
                    nc.scalar.activation(
                        qT[i][:, qc * QCH : (qc + 1) * QCH], pq[:],
                        AF.Identity, bias=bqt[i][:, 0:1],
                    )

        def emit_v_chunk(tcx):
            kk = 128 if tcx < 16 else KREM
            pvt = psv.tile([128, QCH], F32, tag="pq", name=f"pv{tcx}")
            pv = pvt[:, 0:GD]
            for dd in range(8):
                nc.tensor.matmul(
                    pv[:kk, :],
                    xt[dd][:, tcx * 128 : tcx * 128 + kk],
                    w_sb["v"][:, dd * GD : (dd + 1) * GD],
                    start=(dd == 0),
                    stop=(dd == 7),
                )
            c0 = tcx * VBLK
            for pr in range(2):
                for hh in range(2):
                    h = pr * 2 + hh
                    nc.vector.tensor_tensor(
                        vp[pr][:kk, c0 + hh * 65 : c0 + hh * 65 + 64],
                        pv[:kk, h * 64 : (h + 1) * 64],
                        bv_bc[:kk, h * 64 : (h + 1) * 64],
                        op=ALU.add,
                    )

        # ---------------- out-projection ----------------
        def emit_y(qc, onTs):
            # py[q, D] = onT.T @ wo + 1 x bo ; DMA straight from PSUM
            for ts_ in range(4):
                for dc in range(2):
                    py = pyp.tile([128, QCH], F32, tag="pq", name=f"py_{qc}_{ts_}_{dc}")
                    for pr in range(2):
                        nc.tensor.matmul(
                            py[:],
                            onTs[pr][:, ts_ * 128 : (ts_ + 1) * 128],
                            wo_sb[pr][:, dc * QCH : (dc + 1) * QCH],
                            start=(pr == 0), stop=False,
                        )
                    nc.tensor.matmul(
                        py[:],
                        ones_r[:, 0:128],
                        bo_r[:, dc * QCH : (dc + 1) * QCH],
                        start=False, stop=True,
                    )
                    ys = ysp.tile([128, QCH], F32, tag="ys", name=f"ys_{qc}_{ts_}_{dc}")
                    if dc == 0:
                        nc.scalar.activation(ys[:], py[:], AF.Identity)
                    else:
                        nc.vector.tensor_copy(ys[:], py[:])
                    nc.sync.dma_start(
                        dr["y"][qc * QCH + ts_ * 128 : qc * QCH + (ts_ + 1) * 128,
                                dc * QCH : (dc + 1) * QCH],
                        ys[:],
                    )

        # ---------------- attention ----------------
        def att_unit(qc, with_v):
            onTs = []
            for pr in range(2):
                VA = avp.tile([128, QCH], F32, tag="av", name=f"VA_{qc}_{pr}")
                VB = avp.tile([128, QCH], F32, tag="av", name=f"VB_{qc}_{pr}")
                for kc in range(NKC):
                    if with_v and pr == 0:
                        emit_v_chunk(kc)
                    kk = 128 if kc < 16 else KREM
                    Sps = scp.tile([128, 2 * QCH], F32, tag="s", name=f"S_{qc}_{pr}_{kc}")
                    PT = ptp.tile([128, 2 * QCH], F16, tag="pt", name=f"PT_{qc}_{pr}_{kc}")
                    nc.tensor.matmul(
                        Sps[:kk, 0:QCH],
                        kT[pr][0:64, kc * 128 : kc * 128 + kk],
                        qT[pr][0:64, qc * QCH : (qc + 1) * QCH],
                        start=True, stop=True,
                    )
                    nc.tensor.matmul(
                        Sps[:kk, QCH : 2 * QCH],
                        kT[pr][64:128, kc * 128 : kc * 128 + kk],
                        qT[pr][64:128, qc * QCH : (qc + 1) * QCH],
                        start=True, stop=True,
                    )
                    if kc == NKC - 1 or (N_SCHRAU > 0 and kc % (16 // max(N_SCHRAU, 1)) == 2):
                        # Schraudolph exp on DVE (sink chunk + optional tail)
                        with nc.allow_low_precision(reason="fp16 softmax weights"):
                            nc.vector.tensor_scalar(
                                PT[:kk, :].bitcast(I16),
                                Sps[:kk, :],
                                SCHRAU_A, SCHRAU_B,
                                op0=ALU.mult, op1=ALU.add,
                            )
                    else:
                        nc.scalar.activation(PT[:kk, :], Sps[:kk, :], AF.Exp)
                    st, stp = kc == 0, kc == NKC - 1
                    c0 = kc * VBLK
                    nc.tensor.matmul(
                        VA[0:65, :], vp[pr][:kk, c0 : c0 + 65],
                        PT[:kk, 0:QCH], start=st, stop=stp,
                    )
                    nc.tensor.matmul(
                        VB[0:65, :], vp[pr][:kk, c0 + 65 : c0 + VBLK],
                        PT[:kk, QCH : 2 * QCH], start=st, stop=stp,
                    )
                onT = onp.tile([128, QCH], F16, tag="onT", name=f"onT_{qc}_{pr}")
                for hh, V in ((0, VA), (1, VB)):
                    rc = sp.tile([1, QCH], F32, tag="rc", name=f"rc_{qc}_{pr}_{hh}")
                    with nc.allow_low_precision(reason="softmax denom reciprocal"):
                        if RECIP_FAST:
                            dn = sp.tile([1, QCH], F32, tag="dn", name=f"dn_{qc}_{pr}_{hh}")
                            nc.vector.tensor_copy(dn[:], V[64:65, :])
                            nc.vector.reciprocal_approx_fast(rc[:], dn[:])
                        else:
                            nc.vector.reciprocal(rc[:], V[64:65, :])
                    bc = sp.tile([64, QCH], F32, tag="bc", name=f"bc_{qc}_{pr}_{hh}")
                    nc.gpsimd.partition_broadcast(bc[:], rc[:])
                    with nc.allow_low_precision(reason="fp16 attn output"):
                        nc.vector.tensor_tensor(
                            onT[hh * 64 : hh * 64 + 64, :],
                            V[0:64, :], bc[:], op=ALU.mult,
                        )
                onTs.append(onT)
            return onTs

        emit_k()
        emit_q([0])
        y_pend = None
        for qc in range(NQC):
            onTs = att_unit(qc, with_v=(qc == 0))
            if y_pend is not None:
                emit_y(*y_pend)
            y_pend = (qc, onTs)
            if qc == 0:
                emit_q([1, 2, 3])
        emit_y(*y_pend)


def _build_program(reps=1):
    import concourse.bass as bass  # noqa: F401
    import concourse.mybir as mybir
    import concourse.tile as tile
    from concourse import bacc

    F32R = mybir.dt.float32r
    F32 = mybir.dt.float32
    F16 = mybir.dt.float16

    nc = bacc.Bacc("TRN2", num_devices=NCORES)
    dr = {
        "xT": nc.dram_tensor("xT", [D, T], F16, kind="ExternalInput"),
        "wq_p": nc.dram_tensor("wq_p", [128, 8 * GD], F16, kind="ExternalInput"),
        "wk_p": nc.dram_tensor("wk_p", [128, 8 * GD], F16, kind="ExternalInput"),
        "wv_p": nc.dram_tensor("wv_p", [128, 8 * GD], F16, kind="ExternalInput"),
        "wo_t": nc.dram_tensor("wo_t", [GD, D], F16, kind="ExternalInput"),
        "bq": nc.dram_tensor("bq", [GD, 1], F32, kind="ExternalInput"),
        "bk": nc.dram_tensor("bk", [GD, 1], F32, kind="ExternalInput"),
        "bv": nc.dram_tensor("bv", [1, GD], F32, kind="ExternalInput"),
        "bo": nc.dram_tensor("bo", [1, D], F16, kind="ExternalInput"),
        "y": nc.dram_tensor("y", [S, D], F32, kind="ExternalOutput"),
    }

    with tile.TileContext(nc) as tc:
        with tc.tile_pool(name="persist", bufs=1) as pp:
            qT = [pp.tile([128, S], F16, tag=f"qT{i}", name=f"qT{i}") for i in range(2)]
            kT = [pp.tile([128, T], F16, tag=f"kT{i}", name=f"kT{i}") for i in range(2)]
            vp = [pp.tile([128, NKC * VBLK], F16, tag=f"vp{i}", name=f"vp{i}") for i in range(2)]
            wo_sb = [pp.tile([128, D], F16, tag=f"wo{i}", name=f"wo{i}") for i in range(2)]
            bqt = [pp.tile([128, 1], F32, tag=f"bq{i}", name=f"bq{i}") for i in range(2)]
            bkt = [pp.tile([128, 1], F32, tag=f"bk{i}", name=f"bk{i}") for i in range(2)]
            for i in range(2):
                nc.scalar.dma_start(bqt[i][:], dr["bq"][i * 128 : (i + 1) * 128, :])
                nc.scalar.dma_start(bkt[i][:], dr["bk"][i * 128 : (i + 1) * 128, :])
            bv_sb = pp.tile([1, GD], F32, tag="bv")
            nc.scalar.dma_start(bv_sb[:], dr["bv"][:])
            bv_bc = pp.tile([128, GD], F32, tag="bvbc")
            nc.gpsimd.partition_broadcast(bv_bc[:], bv_sb[:])
            # bo + ones row for the bias matmul in the out-projection
            bo_r = pp.tile([1, D], F16, tag="bo")
            nc.scalar.dma_start(bo_r[:], dr["bo"][:])
            ones_r = pp.tile([1, 128], F16, tag="ones_r")
            nc.vector.memset(ones_r[:], 1.0)
            # ones columns of v' (value columns overwritten later)
            for i in range(2):
                nc.vector.memset(vp[i][:], 1.0)
            pers = (qT, kT, vp, wo_sb, bqt, bkt, bv_bc, bo_r, ones_r)
            for _rep in range(reps):
                _emit_body(nc, tc, tile, mybir, dr, pers)
    nc.compile()
    return nc


def _get_program(reps=1):
    key = f"nc{reps}_{N_SCHRAU}_{RECIP_FAST}_{PB_BCAST}"
    if key not in _prog_cache:
        _prog_cache[key] = _build_program(reps)
    return _prog_cache[key]


def _pack(a):
    # [1024, 256] -> [128, 2048]: dd-th 128-row block becomes column block dd
    return np.concatenate([a[dd * 128 : (dd + 1) * 128] for dd in range(8)], axis=1)


def _host_inputs(x, sink_tokens, wq, bq, wk, bk, wv, bv, wo, bo):
    f = np.float32
    h = np.float16
    x = np.asarray(x, f)
    sink = np.asarray(sink_tokens, f)[0]            # [NS, D]
    wq, wk, wv, wo = (np.asarray(a, f) for a in (wq, wk, wv, wo))
    bq, bk, bv, bo = (np.asarray(a, f) for a in (bq, bk, bv, bo))
    sc = np.float32(1.0 / np.sqrt(HD))
    in_maps = []
    for core in range(NCORES):
        b, g = core // 4, core % 4
        xs = np.concatenate([x[b], sink], axis=0)   # sinks LAST
        xT = np.ascontiguousarray(xs.T).astype(h)
        sl = slice(g * GD, (g + 1) * GD)
        in_maps.append({
            "xT": xT,
            "wq_p": _pack(np.ascontiguousarray(wq[sl].T) * sc).astype(h),
            "wk_p": _pack(np.ascontiguousarray(wk[sl].T)).astype(h),
            "wv_p": _pack(np.ascontiguousarray(wv[sl].T)).astype(h),
            "wo_t": np.ascontiguousarray(wo[:, sl].T).astype(h),
            "bq": (bq[sl] * sc).reshape(GD, 1).copy(),
            "bk": bk[sl].reshape(GD, 1).copy(),
            "bv": bv[sl].reshape(1, GD).copy(),
            "bo": (bo if g == 0 else np.zeros_like(bo)).reshape(1, D).astype(h),
        })
    return in_maps


def kernel(x, sink_tokens, wq, bq, wk, bk, wv, bv, wo, bo):
    from concourse.bass_utils import run_bass_kernel_spmd

    nc = _get_program()
    in_maps = _host_inputs(x, sink_tokens, wq, bq, wk, bk, wv, bv, wo, bo)
    res = None
    last_exc = None
    for attempt in range(3):
        try:
            res = run_bass_kernel_spmd(nc, in_maps, core_ids=list(range(NCORES)))
            break
        except Exception as e:  # transient NRT/axon failures: retry
            last_exc = e
            import time as _time
            _time.sleep(2.0 * (attempt + 1))
    if res is None:
        raise last_exc
    y = np.zeros((B, S, D), np.float64)
    for core in range(NCORES):
        y[core // 4] += res.results[core]["y"]
    return y.astype(np.float32)
